# revision 1
# baseline (speedup 1.0000x reference)
"""Causal self-attention (B=4, S=2048, D=1024, H=16, hd=64) on 8 TRN2 cores.

Sharding: core c = (batch b = c//2, head-group g = c%2); each core computes
8 heads for one batch. Out-projection partials are summed on host (the only
cross-shard reduction).

Device kernel layout (all matmul contractions have the contracted dim on
SBUF partitions; everything stays transposed so no on-device transposes):
  qT,kT  [64*2heads, S]  = wqkvT-chunk.T @ xT          (stationary weights)
  v_aug  [S-block, 8*65] = xT-chunk.T @ wvT (+ ones col per head for sums)
  sT     [j 128, i 512]  = kT-slice.T @ qT-slice        (2 heads row-packed)
  pT     = exp(sT/8)  bf16 via ACT; diagonal blocks masked by affine_select
  outT   [65, i]        += v_aug.T @ pT   (row 64 accumulates softmax sums)
  attnT  = outT * bcast(1/sums)           (gpsimd partition_broadcast)
  out    [s 128, e]      = attnT-chunk.T @ woutT-chunk  (accum over c-chunks)

QKV work for pair p+1 is emitted between attention pairs so the PE always
has independent matmuls while ACT runs exp (keeps HAM at full clock).
"""
import sys
import os

sys.path.insert(0, "/opt/trn_rl_repo")

import numpy as np
import ml_dtypes
from contextlib import ExitStack

S = 2048
D = 1024
HL = 8          # heads per core
HD = 64
PAIRS = 4       # head pairs per core
NIB = 4         # i-blocks of 512
N_CORES = 8

_CACHE = {}
LAST_EXEC_TIME_NS = None


def _build():
    import concourse.tile as tile
    import concourse.mybir as mybir
    from concourse import bacc

    bf = mybir.dt.bfloat16
    f32 = mybir.dt.float32
    EXP = mybir.ActivationFunctionType.Exp
    GE = mybir.AluOpType.is_ge

    nc = bacc.Bacc("TRN2", target_bir_lowering=False, debug=False,
                   num_devices=N_CORES)
    xT_d = nc.dram_tensor("xT", [D, S], bf, kind="ExternalInput").ap()
    wqkvT_d = nc.dram_tensor("wqkvT", [D, 3 * 512], bf,
                             kind="ExternalInput").ap()
    woutT_d = nc.dram_tensor("woutT", [512, D], bf, kind="ExternalInput").ap()
    out_d = nc.dram_tensor("out", [S, D], f32, kind="ExternalOutput").ap()

    with tile.TileContext(nc) as tc, ExitStack() as ctx:
        sb = ctx.enter_context(tc.tile_pool(name="sb", bufs=1))
        # PSUM: "mm" = 2x [128,1024] (score batches), "ps5" = 4x [128,512]
        # (qkv accum, AV accum, out-proj accum) -> 8 banks total.
        mm = ctx.enter_context(tc.tile_pool(name="mm", bufs=2, space="PSUM"))
        ps5 = ctx.enter_context(tc.tile_pool(name="ps5", bufs=4,
                                             space="PSUM"))
        pp = ctx.enter_context(tc.tile_pool(name="pp", bufs=8))
        rsp = ctx.enter_context(tc.tile_pool(name="rsp", bufs=4))
        bcsp = ctx.enter_context(tc.tile_pool(name="bcsp", bufs=4))
        osbp = ctx.enter_context(tc.tile_pool(name="osbp", bufs=3))

        # ---- persistent SBUF tiles -------------------------------------
        xt = [sb.tile([128, S], bf, tag=f"xt{d}", name=f"xt{d}")
              for d in range(8)]
        wqkv = [sb.tile([128, 1536], bf, tag=f"wqkv{d}", name=f"wqkv{d}")
                for d in range(8)]
        wout = [sb.tile([128, D], bf, tag=f"wout{c}", name=f"wout{c}")
                for c in range(4)]
        qT = [sb.tile([128, S], bf, tag=f"qT{p}", name=f"qT{p}")
              for p in range(PAIRS)]
        kT = [sb.tile([128, S], bf, tag=f"kT{p}", name=f"kT{p}")
              for p in range(PAIRS)]
        vaug = [sb.tile([128, HL, HD + 1], bf, tag=f"vaug{s}",
                        name=f"vaug{s}") for s in range(16)]
        attnT = [sb.tile([128, S], bf, tag=f"attnT{p}", name=f"attnT{p}")
                 for p in range(PAIRS)]

        for d in range(8):
            nc.sync.dma_start(xt[d][:], xT_d[128 * d:128 * (d + 1), :])
            nc.sync.dma_start(wqkv[d][:], wqkvT_d[128 * d:128 * (d + 1), :])
        for c in range(4):
            nc.sync.dma_start(wout[c][:], woutT_d[128 * c:128 * (c + 1), :])
        for s in range(16):
            nc.gpsimd.memset(vaug[s][:], 1.0)
        # causal masks for the 4 diagonal offsets: keep where i >= 128*m + j
        masks = [sb.tile([128, 512], bf, tag=f"mask{m}", name=f"mask{m}")
                 for m in range(4)]
        for m in range(4):
            nc.gpsimd.memset(masks[m][:], 1.0)
            nc.gpsimd.affine_select(
                out=masks[m][:], in_=masks[m][:], compare_op=GE, fill=0.0,
                base=-128 * m, channel_multiplier=-1, pattern=[[1, 512]])

        # ---- emission helpers ------------------------------------------
        def emit_v(sblk):
            ps = ps5.tile([128, 512], f32, tag="ps5", name=f"vps{sblk}")
            for dc in range(8):
                nc.tensor.matmul(ps[:],
                                 lhsT=xt[dc][:, 128 * sblk:128 * (sblk + 1)],
                                 rhs=wqkv[dc][:, 1024:1536],
                                 start=(dc == 0), stop=(dc == 7))
            nc.scalar.copy(
                vaug[sblk][:, :, 0:64],
                ps[:].rearrange("p (h d) -> p h d", h=HL))

        def emit_qk(pair):
            # nb = pair -> q columns, nb = pair + 4 -> k columns
            for nb in (pair, pair + 4):
                dest = qT[pair] if nb < 4 else kT[pair]
                for sc in range(4):
                    ps = ps5.tile([128, 512], f32, tag="ps5",
                                  name=f"qkps{nb}_{sc}")
                    for dc in range(8):
                        nc.tensor.matmul(
                            ps[:],
                            lhsT=wqkv[dc][:, 128 * nb:128 * (nb + 1)],
                            rhs=xt[dc][:, 512 * sc:512 * (sc + 1)],
                            start=(dc == 0), stop=(dc == 7))
                    nc.vector.tensor_copy(dest[:, 512 * sc:512 * (sc + 1)],
                                          ps[:])

        def emit_qkexp(pair, ib, jb):
            off = max(0, 128 * (jb - 4 * ib))
            s2 = mm.tile([128, 1024], f32, tag="mm",
                         name=f"s2_{pair}{ib}{jb}")
            for h01 in range(2):
                r0, r1 = 64 * h01, 64 * (h01 + 1)
                nc.tensor.matmul(
                    s2[:, 512 * h01 + off:512 * (h01 + 1)],
                    lhsT=kT[pair][r0:r1, 128 * jb:128 * (jb + 1)],
                    rhs=qT[pair][r0:r1, 512 * ib + off:512 * (ib + 1)],
                    start=True, stop=True)
            pX = pp.tile([128, 1024], bf, tag="pp", name=f"pX{pair}{ib}{jb}")
            s3 = s2[:].rearrange("p (h i) -> p h i", h=2)
            p3 = pX[:].rearrange("p (h i) -> p h i", h=2)
            nc.scalar.activation(p3[:, :, off:512], s3[:, :, off:512],
                                 EXP, scale=0.125)
            if jb >= 4 * ib:
                m = jb - 4 * ib
                nc.vector.tensor_mul(
                    p3[:, :, off:512], p3[:, :, off:512],
                    masks[m][:, off:512].unsqueeze(1).broadcast_to(
                        [128, 2, 512 - off]))
            return pX

        def emit_attn(pair, only_ib=None, pre_px=None):
            for ib in range(NIB) if only_ib is None else [only_ib]:
                n_jb = 4 * (ib + 1)
                oA = ps5.tile([65, 512], f32, tag="ps5", name=f"oA{pair}{ib}")
                oB = ps5.tile([65, 512], f32, tag="ps5", name=f"oB{pair}{ib}")
                for jb in range(n_jb):
                    off = max(0, 128 * (jb - 4 * ib))
                    if pre_px is not None and jb in pre_px:
                        pX = pre_px[jb]
                    else:
                        pX = emit_qkexp(pair, ib, jb)
                    for h01, oX in ((0, oA), (1, oB)):
                        nc.tensor.matmul(
                            oX[:, off:512],
                            lhsT=vaug[jb][:, 2 * pair + h01, :],
                            rhs=pX[:, 512 * h01 + off:512 * (h01 + 1)],
                            start=(jb == 0), stop=(jb == n_jb - 1))
                for h01, oX in ((0, oA), (1, oB)):
                    tmp = rsp.tile([1, 512], f32, tag="rtmp",
                                   name=f"rt{pair}{ib}{h01}")
                    nc.vector.tensor_copy(tmp[:], oX[64:65, :])
                    rs = rsp.tile([1, 512], f32, tag="rsp",
                                  name=f"rs{pair}{ib}{h01}")
                    nc.vector.reciprocal_approx_fast(rs[:], tmp[:])
                    bcs = bcsp.tile([64, 512], f32, tag="bcsp",
                                    name=f"bcs{pair}{ib}{h01}")
                    nc.gpsimd.partition_broadcast(bcs[:], rs[:])
                    nc.vector.tensor_mul(
                        attnT[pair][64 * h01:64 * (h01 + 1),
                                    512 * ib:512 * (ib + 1)],
                        oX[0:64, :], bcs[:])

        def emit_outproj(sblk):
            osb = osbp.tile([128, D], f32, tag="osbp", name=f"osb{sblk}")
            for eh in range(2):
                ps = ps5.tile([128, 512], f32, tag="ps5",
                              name=f"ops{sblk}{eh}")
                for cc in range(4):
                    nc.tensor.matmul(
                        ps[:],
                        lhsT=attnT[cc][:, 128 * sblk:128 * (sblk + 1)],
                        rhs=wout[cc][:, 512 * eh:512 * (eh + 1)],
                        start=(cc == 0), stop=(cc == 3))
                nc.scalar.copy(osb[:, 512 * eh:512 * (eh + 1)], ps[:])
            nc.sync.dma_start(out_d[128 * sblk:128 * (sblk + 1), :], osb[:])

        # ---- emission order (== program order for tile deps): vaug[s]
        # must be written before the attention ib that reads it; attnT
        # before the out-proj s-blocks that read it. exp work starts as
        # early as possible; out-proj interleaves with the last pair. -----
        emit_qk(0)
        # ib0 of pair 0: QK+exp emitted before the v-phase so ACT starts
        # as early as possible (AV waits for vaug, exp does not)
        pre = {jb: emit_qkexp(0, 0, jb) for jb in range(4)}
        for sblk in range(4):
            emit_v(sblk)
        emit_attn(0, only_ib=0, pre_px=pre)
        for ib in range(1, NIB):
            for sblk in range(4 * ib, 4 * ib + 4):
                emit_v(sblk)
            emit_attn(0, only_ib=ib)
        emit_qk(1)
        emit_attn(1)
        emit_qk(2)
        emit_attn(2)
        emit_qk(3)
        for ib in range(NIB):
            emit_attn(3, only_ib=ib)
            for sblk in range(4 * ib, 4 * ib + 4):
                emit_outproj(sblk)

    nc.compile()
    return nc


def _get_nc():
    if "nc" not in _CACHE:
        _CACHE["nc"] = _build()
    return _CACHE["nc"]


def _shard_inputs(x, w_qkv, w_out):
    bf = ml_dtypes.bfloat16
    in_maps = []
    for c in range(N_CORES):
        b, g = divmod(c, 2)
        xT = np.ascontiguousarray(x[b].T).astype(bf)
        wq = w_qkv[512 * g:512 * (g + 1)]
        wk = w_qkv[1024 + 512 * g:1024 + 512 * (g + 1)]
        wv = w_qkv[2048 + 512 * g:2048 + 512 * (g + 1)]
        wqkvT = np.ascontiguousarray(
            np.concatenate([wq, wk, wv], axis=0).T).astype(bf)
        woutT = np.ascontiguousarray(w_out[:, 512 * g:512 * (g + 1)].T
                                     ).astype(bf)
        in_maps.append({"xT": xT, "wqkvT": wqkvT, "woutT": woutT})
    return in_maps


def kernel(x, w_qkv, w_out):
    global LAST_EXEC_TIME_NS
    from concourse.bass_utils import run_bass_kernel_spmd

    nc = _get_nc()
    in_maps = _shard_inputs(np.asarray(x, dtype=np.float32),
                            np.asarray(w_qkv, dtype=np.float32),
                            np.asarray(w_out, dtype=np.float32))
    trace = bool(int(os.environ.get("KBENCH_TRACE", "0")))
    res = run_bass_kernel_spmd(nc, in_maps, list(range(N_CORES)), trace=trace)
    LAST_EXEC_TIME_NS = res.exec_time_ns
    out = np.empty((4, S, D), dtype=np.float32)
    for b in range(4):
        out[b] = res.results[2 * b]["out"] + res.results[2 * b + 1]["out"]
    return out



# revision 2
# speedup vs baseline: 1.1045x; 1.1045x over previous
"""Causal self-attention (B=4, S=2048, D=1024, H=16, hd=64) on 8 TRN2 cores.

Sharding: core c = (batch b = c//2, head-group g = c%2); each core computes
8 heads for one batch. Out-projection partials are summed on host (the only
cross-shard reduction).

Schedule: the score->exp->AV pipeline is the ACT-bound critical path; all
other PE work (next-pair QKV chains, v-projection, out-projection) is woven
into it as fine-grained "filler" half-units popped from a global queue, one
per (score, AV) slot, so the PE never drains (keeps the tensor engine at its
max p-state) and the ACT engine always has exp work queued.  Causal masks
are applied in-place on the exp output by gpsimd.affine_select (no mask
tiles, no DVE mask-mul).  PSUM: 2x[128,1024] score tiles (4 banks),
3x[65,512] rotating AV accumulators (3 banks; 3-deep so ib boundaries don't
stall), 1x[128,512] shared by all filler chains.  Output is written bf16
and summed on host in f32.
"""
import sys
import os
from collections import deque

sys.path.insert(0, "/opt/trn_rl_repo")

import numpy as np
import ml_dtypes
from contextlib import ExitStack

S = 2048
D = 1024
HL = 8          # heads per core
HD = 64
PAIRS = 4       # head pairs per core
NIB = 4         # i-blocks of 512
N_CORES = 8

_CACHE = {}
LAST_EXEC_TIME_NS = None


def _build():
    import concourse.tile as tile
    import concourse.mybir as mybir
    from concourse import bacc

    bf = mybir.dt.bfloat16
    f32 = mybir.dt.float32
    EXP = mybir.ActivationFunctionType.Exp
    GE = mybir.AluOpType.is_ge

    nc = bacc.Bacc("TRN2", target_bir_lowering=False, debug=False,
                   num_devices=N_CORES)
    xT_d = nc.dram_tensor("xT", [D, S], bf, kind="ExternalInput").ap()
    wqkvT_d = nc.dram_tensor("wqkvT", [D, 3 * 512], bf,
                             kind="ExternalInput").ap()
    woutT_d = nc.dram_tensor("woutT", [512, D], bf, kind="ExternalInput").ap()
    out_d = nc.dram_tensor("out", [S, D], bf, kind="ExternalOutput").ap()

    with tile.TileContext(nc) as tc, ExitStack() as ctx:
        sb = ctx.enter_context(tc.tile_pool(name="sb", bufs=1))
        mm = ctx.enter_context(tc.tile_pool(name="mm", bufs=2, space="PSUM"))
        av = ctx.enter_context(tc.tile_pool(name="av", bufs=3, space="PSUM"))
        ps5 = ctx.enter_context(tc.tile_pool(name="ps5", bufs=1,
                                             space="PSUM"))
        pp = ctx.enter_context(tc.tile_pool(name="pp", bufs=8))
        rsp = ctx.enter_context(tc.tile_pool(name="rsp", bufs=4))
        bcsp = ctx.enter_context(tc.tile_pool(name="bcsp", bufs=4))
        osbp = ctx.enter_context(tc.tile_pool(name="osbp", bufs=3))

        # ---- persistent SBUF tiles -------------------------------------
        xt = [sb.tile([128, S], bf, tag=f"xt{d}", name=f"xt{d}")
              for d in range(8)]
        wqkv = [sb.tile([128, 1536], bf, tag=f"wqkv{d}", name=f"wqkv{d}")
                for d in range(8)]
        wout = [sb.tile([128, D], bf, tag=f"wout{c}", name=f"wout{c}")
                for c in range(4)]
        qT = [sb.tile([128, S], bf, tag=f"qT{p}", name=f"qT{p}")
              for p in range(PAIRS)]
        kT = [sb.tile([128, S], bf, tag=f"kT{p}", name=f"kT{p}")
              for p in range(PAIRS)]
        vaug = [sb.tile([128, HL, HD + 1], bf, tag=f"vaug{s}",
                        name=f"vaug{s}") for s in range(16)]
        attnT = [sb.tile([128, S], bf, tag=f"attnT{p}", name=f"attnT{p}")
                 for p in range(PAIRS)]

        # ---- DMAs, priority-ordered and striped ------------------------
        # 1) pair-0 q and k weight columns (small, unblock first chains)
        for dc in range(8):
            nc.sync.dma_start(wqkv[dc][:, 0:128],
                              wqkvT_d[128 * dc:128 * (dc + 1), 0:128])
        for dc in range(8):
            nc.sync.dma_start(wqkv[dc][:, 512:640],
                              wqkvT_d[128 * dc:128 * (dc + 1), 512:640])
        # 2) x columns 0:512 (first qk chains + v sblk 0..3), striped x2
        for dc in range(8):
            for h in range(2):
                nc.sync.dma_start(
                    xt[dc][:, 256 * h:256 * (h + 1)],
                    xT_d[128 * dc:128 * (dc + 1), 256 * h:256 * (h + 1)])
        # 3) v weight columns (striped x2)
        for dc in range(8):
            for h in range(2):
                nc.sync.dma_start(
                    wqkv[dc][:, 1024 + 256 * h:1024 + 256 * (h + 1)],
                    wqkvT_d[128 * dc:128 * (dc + 1),
                            1024 + 256 * h:1024 + 256 * (h + 1)])
        # 4) rest of x, in sc order
        for sc in range(1, 4):
            for dc in range(8):
                nc.sync.dma_start(
                    xt[dc][:, 512 * sc:512 * (sc + 1)],
                    xT_d[128 * dc:128 * (dc + 1), 512 * sc:512 * (sc + 1)])
        # 5) remaining q/k weight columns (pairs 1..3)
        for dc in range(8):
            nc.sync.dma_start(wqkv[dc][:, 128:512],
                              wqkvT_d[128 * dc:128 * (dc + 1), 128:512])
        for dc in range(8):
            nc.sync.dma_start(wqkv[dc][:, 640:1024],
                              wqkvT_d[128 * dc:128 * (dc + 1), 640:1024])
        # 6) out-proj weights (needed only in pair 3)
        for c in range(4):
            nc.sync.dma_start(wout[c][:], woutT_d[128 * c:128 * (c + 1), :])
        # ones column for the softmax-sum row of the AV matmul
        for s in range(16):
            nc.gpsimd.memset(vaug[s][:, :, 64:65], 1.0)

        # ---- filler-unit queue ------------------------------------------
        queue = deque()          # (uid, closure)
        remaining = {}           # uid -> halves left to emit

        def push_unit(uid, halves):
            remaining[uid] = len(halves)
            for h in halves:
                queue.append((uid, h))

        def pop_one():
            if queue:
                uid, h = queue.popleft()
                h()
                remaining[uid] -= 1

        def drain_until(uid):
            while remaining.get(uid, 0) > 0:
                u2, h = queue.popleft()
                h()
                remaining[u2] -= 1

        def drain_all():
            while queue:
                pop_one()

        # ---- unit builders ----------------------------------------------
        def qk_unit(pair, qk, sc):
            # qk: 0 -> q (wqkv cols 128*pair), 1 -> k (cols 512+128*pair)
            nb = pair if qk == 0 else 4 + pair
            dest = qT[pair] if qk == 0 else kT[pair]
            st = {}

            def h1():
                ps = ps5.tile([128, 512], f32, tag="ps5",
                              name=f"qk{pair}_{qk}_{sc}")
                st["ps"] = ps
                for dc in range(4):
                    nc.tensor.matmul(
                        ps[:], lhsT=wqkv[dc][:, 128 * nb:128 * (nb + 1)],
                        rhs=xt[dc][:, 512 * sc:512 * (sc + 1)],
                        start=(dc == 0), stop=False)

            def h2():
                ps = st["ps"]
                for dc in range(4, 8):
                    nc.tensor.matmul(
                        ps[:], lhsT=wqkv[dc][:, 128 * nb:128 * (nb + 1)],
                        rhs=xt[dc][:, 512 * sc:512 * (sc + 1)],
                        start=False, stop=(dc == 7))
                nc.vector.tensor_copy(dest[:, 512 * sc:512 * (sc + 1)],
                                      ps[:])
            return [h1, h2]

        def v_unit(sblk):
            st = {}

            def h1():
                ps = ps5.tile([128, 512], f32, tag="ps5", name=f"v{sblk}")
                st["ps"] = ps
                for dc in range(4):
                    nc.tensor.matmul(
                        ps[:], lhsT=xt[dc][:, 128 * sblk:128 * (sblk + 1)],
                        rhs=wqkv[dc][:, 1024:1536],
                        start=(dc == 0), stop=False)

            def h2():
                ps = st["ps"]
                for dc in range(4, 8):
                    nc.tensor.matmul(
                        ps[:], lhsT=xt[dc][:, 128 * sblk:128 * (sblk + 1)],
                        rhs=wqkv[dc][:, 1024:1536],
                        start=False, stop=(dc == 7))
                nc.vector.tensor_copy(
                    vaug[sblk][:, :, 0:64],
                    ps[:].rearrange("p (h d) -> p h d", h=HL))
            return [h1, h2]

        def op_unit(sblk):
            st = {}

            def half(eh):
                def h():
                    if eh == 0:
                        st["osb"] = osbp.tile([128, D], bf, tag="osbp",
                                              name=f"osb{sblk}")
                    osb = st["osb"]
                    ps = ps5.tile([128, 512], f32, tag="ps5",
                                  name=f"op{sblk}_{eh}")
                    for cc in range(4):
                        nc.tensor.matmul(
                            ps[:],
                            lhsT=attnT[cc][:, 128 * sblk:128 * (sblk + 1)],
                            rhs=wout[cc][:, 512 * eh:512 * (eh + 1)],
                            start=(cc == 0), stop=(cc == 3))
                    nc.vector.tensor_copy(osb[:, 512 * eh:512 * (eh + 1)],
                                          ps[:])
                    if eh == 1:
                        for q in range(2):
                            nc.sync.dma_start(
                                out_d[128 * sblk:128 * (sblk + 1),
                                      512 * q:512 * (q + 1)],
                                osb[:, 512 * q:512 * (q + 1)])
                return h
            return [half(0), half(1)]

        # ---- attention emitters -----------------------------------------
        def emit_score(pair, ib, jb):
            off = max(0, 128 * (jb - 4 * ib))
            s2 = mm.tile([128, 1024], f32, tag="mm",
                         name=f"s2_{pair}{ib}{jb}")
            for h01 in range(2):
                r0, r1 = 64 * h01, 64 * (h01 + 1)
                nc.tensor.matmul(
                    s2[:, 512 * h01 + off:512 * (h01 + 1)],
                    lhsT=kT[pair][r0:r1, 128 * jb:128 * (jb + 1)],
                    rhs=qT[pair][r0:r1, 512 * ib + off:512 * (ib + 1)],
                    start=True, stop=True)
            pX = pp.tile([128, 1024], bf, tag="pp", name=f"pX{pair}{ib}{jb}")
            s3 = s2[:].rearrange("p (h i) -> p h i", h=2)
            p3 = pX[:].rearrange("p (h i) -> p h i", h=2)
            nc.scalar.activation(p3[:, :, off:512], s3[:, :, off:512],
                                 EXP, scale=0.125)
            if jb >= 4 * ib:
                # in-place causal wedge: keep where i_rel - j >= 0
                nc.gpsimd.affine_select(
                    out=p3[:, :, off:512], in_=p3[:, :, off:512],
                    compare_op=GE, fill=0.0, base=0, channel_multiplier=-1,
                    pattern=[[0, 2], [1, 512 - off]])
            return pX

        def emit_av(pair, ib, jb, pX, oA, oB):
            off = max(0, 128 * (jb - 4 * ib))
            n_jb = 4 * (ib + 1)
            for h01, oX in ((0, oA), (1, oB)):
                nc.tensor.matmul(
                    oX[:, off:512],
                    lhsT=vaug[jb][:, 2 * pair + h01, :],
                    rhs=pX[:, 512 * h01 + off:512 * (h01 + 1)],
                    start=(jb == 0), stop=(jb == n_jb - 1))

        def emit_norm(pair, ib, oA, oB):
            for h01, oX in ((0, oA), (1, oB)):
                tmp = rsp.tile([1, 512], f32, tag="rtmp",
                               name=f"rt{pair}{ib}{h01}")
                nc.vector.tensor_copy(tmp[:], oX[64:65, :])
                rs = rsp.tile([1, 512], f32, tag="rsp",
                              name=f"rs{pair}{ib}{h01}")
                nc.vector.reciprocal_approx_fast(rs[:], tmp[:])
                bcs = bcsp.tile([64, 512], f32, tag="bcsp",
                                name=f"bcs{pair}{ib}{h01}")
                nc.gpsimd.partition_broadcast(bcs[:], rs[:])
                nc.vector.tensor_mul(
                    attnT[pair][64 * h01:64 * (h01 + 1),
                                512 * ib:512 * (ib + 1)],
                    oX[0:64, :], bcs[:])

        # ---- build the global filler queue ------------------------------
        push_unit(("qk", 0, 0, 0), qk_unit(0, 0, 0))
        push_unit(("qk", 0, 1, 0), qk_unit(0, 1, 0))
        for s in range(4):
            push_unit(("v", s), v_unit(s))
        for sc in range(1, 4):
            push_unit(("qk", 0, 0, sc), qk_unit(0, 0, sc))
            push_unit(("qk", 0, 1, sc), qk_unit(0, 1, sc))
            for s in range(4 * sc, 4 * sc + 4):
                push_unit(("v", s), v_unit(s))
        for pair in range(1, 4):
            for sc in range(4):
                push_unit(("qk", pair, 0, sc), qk_unit(pair, 0, sc))
                push_unit(("qk", pair, 1, sc), qk_unit(pair, 1, sc))

        # ---- main emission: 4 pairs, score->exp->AV with weaving --------
        for pair in range(PAIRS):
            for ib in range(NIB):
                drain_until(("qk", pair, 0, ib))
                n_jb = 4 * (ib + 1)
                oA = av.tile([65, 512], f32, tag="av", name=f"oA{pair}{ib}")
                oB = av.tile([65, 512], f32, tag="av", name=f"oB{pair}{ib}")
                pend = None   # delay AV(jb) until after score(jb+1)
                for jb in range(n_jb):
                    drain_until(("qk", pair, 1, jb // 4))
                    if pair == 0:
                        drain_until(("v", jb))
                    pX = emit_score(pair, ib, jb)
                    if pend is not None:
                        emit_av(pair, ib, pend[0], pend[1], oA, oB)
                    pop_one()
                    pend = (jb, pX)
                emit_av(pair, ib, pend[0], pend[1], oA, oB)
                emit_norm(pair, ib, oA, oB)
                if pair == 3:
                    for sblk in range(4 * ib, 4 * ib + 4):
                        push_unit(("op", sblk), op_unit(sblk))
        drain_all()

    nc.compile()
    return nc


def _get_nc():
    if "nc" not in _CACHE:
        _CACHE["nc"] = _build()
    return _CACHE["nc"]


def _shard_inputs(x, w_qkv, w_out):
    bf = ml_dtypes.bfloat16
    in_maps = []
    for c in range(N_CORES):
        b, g = divmod(c, 2)
        xT = np.ascontiguousarray(x[b].T).astype(bf)
        wq = w_qkv[512 * g:512 * (g + 1)]
        wk = w_qkv[1024 + 512 * g:1024 + 512 * (g + 1)]
        wv = w_qkv[2048 + 512 * g:2048 + 512 * (g + 1)]
        wqkvT = np.ascontiguousarray(
            np.concatenate([wq, wk, wv], axis=0).T).astype(bf)
        woutT = np.ascontiguousarray(w_out[:, 512 * g:512 * (g + 1)].T
                                     ).astype(bf)
        in_maps.append({"xT": xT, "wqkvT": wqkvT, "woutT": woutT})
    return in_maps


def kernel(x, w_qkv, w_out):
    global LAST_EXEC_TIME_NS
    from concourse.bass_utils import run_bass_kernel_spmd

    nc = _get_nc()
    in_maps = _shard_inputs(np.asarray(x, dtype=np.float32),
                            np.asarray(w_qkv, dtype=np.float32),
                            np.asarray(w_out, dtype=np.float32))
    trace = bool(int(os.environ.get("KBENCH_TRACE", "0")))
    res = run_bass_kernel_spmd(nc, in_maps, list(range(N_CORES)), trace=trace)
    LAST_EXEC_TIME_NS = res.exec_time_ns
    out = np.empty((4, S, D), dtype=np.float32)
    for b in range(4):
        out[b] = (res.results[2 * b]["out"].astype(np.float32)
                  + res.results[2 * b + 1]["out"].astype(np.float32))
    return out
